# revision 1
# baseline (speedup 1.0000x reference)
"""Multi-head attention (B=2, S=2048, E=1024, H=16) on 8 TRN2 NeuronCores.

Sharding: tensor-parallel on heads — core c computes heads {2c, 2c+1} end to end
(QKV projection slice, attention, and the row-parallel slice of out_proj), and
returns a partial [4096, 1024] output; the host sums the 8 partials and adds
b_out.

Per-core device program (identical on all cores; only input data differs):
  phase 1: qkvT[f, t] = sum_E w_inT[E, f] * xT[E, t] + b_in   (fp32r matmuls)
           feature-major layout [128, 3, 4096]: partitions = 2 heads x 64 dims,
           fb in {q, k, v}.
  phase 2: PE-transpose v -> vT [k, 130] bf16 per (b, kchunk); cols 64/129 are
           ones used to compute softmax sums for free during PV.
  phase 3: per (b, qblock of 1024), per kchunk, both heads row-packed
           concurrently on the PE: scoresT psum [128 k, 1024 q] -> ACT
           exp(0.125*s) -> es bf16; PV: psum[65, q] = [v | 1].T @ es
           accumulated over kchunks (row 64 = softmax sums); normalize via
           reciprocal + gpsimd partition_broadcast + DVE multiply.
  phase 4: out_proj: psum[t, e] accumulates both heads' [64]-contraction
           matmuls; evict + DMA partial out.
"""
import sys

sys.path.insert(0, "/opt/trn_rl_repo")
import numpy as np
import ml_dtypes
import concourse.bass as bass
import concourse.mybir as mybir
import concourse.tile as tile
from concourse import bacc
from concourse.bass_utils import run_bass_kernel_spmd
from concourse.masks import make_identity

P = 128
B = 2
S = 2048
E = 1024
H = 16
D = 64           # head dim
NCORES = 8
T = B * S        # 4096 global tokens
EC = E // P      # 8 contraction chunks for QKV
QB = 1024        # q block size
NQB = S // QB    # q blocks per batch
KC = S // P      # 16 k chunks per batch
TCH = T // P     # 32 token chunks

F32 = mybir.dt.float32
F32R = mybir.dt.float32r
BF16 = mybir.dt.bfloat16

SC_BUFS = 1
_COMPILED = None


def build(repeat=1):
    nc = bacc.Bacc(None, target_bir_lowering=False)
    xT_d = nc.dram_tensor("xT", [P, EC, T], BF16, kind="ExternalInput")
    w_inT_d = nc.dram_tensor("w_inT", [P, EC, 3 * P], BF16, kind="ExternalInput")
    b_in_d = nc.dram_tensor("b_in", [P, 3], F32, kind="ExternalInput")
    w_outT_d = nc.dram_tensor("w_outT", [D, 2, E], BF16, kind="ExternalInput")
    out_d = nc.dram_tensor("out", [TCH, P, E], F32, kind="ExternalOutput")

    TB = 512           # token block for streaming xT
    QQ = 512           # q quarter (psum bank width)

    with tile.TileContext(nc) as tc:
        with (
            tc.tile_pool(name="const", bufs=1) as const,
            tc.tile_pool(name="main", bufs=1) as main,
            tc.tile_pool(name="attn_p", bufs=3) as attn_p,
            tc.tile_pool(name="outp", bufs=3) as outp,
            tc.tile_pool(name="small", bufs=2) as small,
            tc.tile_pool(name="win", bufs=1) as win,
            tc.tile_pool(name="qkv_in", bufs=2) as qkv_in,
            tc.tile_pool(name="es_pool", bufs=10) as es_pool,
            tc.tile_pool(name="psum", bufs=1, space="PSUM") as psum,
        ):
            identity = const.tile([P, P], BF16)
            make_identity(nc, identity)
            b_in_sb = const.tile([P, 3], F32)
            nc.sync.dma_start(b_in_sb[:], b_in_d[:])
            w_outT_sb = const.tile([D, 2, E], BF16)
            nc.sync.dma_start(w_outT_sb[:], w_outT_d[:])
            w_inT_sb = const.tile([P, EC, 3 * P], BF16)
            nc.sync.dma_start(w_inT_sb[:], w_inT_d[:])

            qkT = main.tile([P, 2, T], BF16)           # 16 KB/partition
            vT = main.tile([P, B, KC, 130], BF16)      # ~8 KB/partition
            nc.vector.memset(vT[:, :, :, 64:65], 1.0)
            nc.vector.memset(vT[:, :, :, 129:130], 1.0)

            for _rep in range(repeat):
                for b in range(B):
                    def emit_tb(tbi, b=b):
                        tb = b * (S // TB) + tbi
                        xt = qkv_in.tile([P, EC, TB], BF16, name="xt")
                        nc.sync.dma_start(xt[:], xT_d[:, :, tb * TB:(tb + 1) * TB])
                        v_sb = qkv_in.tile([P, TB], BF16, name="vsb")
                        for fb in range(3):
                            acc = psum.tile([P, TB], F32, name="qkvp", bufs=2)
                            for ec in range(EC):
                                nc.tensor.matmul(
                                    acc[:],
                                    w_inT_sb[:, ec, fb * P:(fb + 1) * P],
                                    xt[:, ec, :],
                                    start=(ec == 0),
                                    stop=(ec == EC - 1),
                                )
                            nc.vector.tensor_scalar(
                                out=(qkT[:, fb, tb * TB:(tb + 1) * TB] if fb < 2
                                     else v_sb[:]),
                                in0=acc[:],
                                scalar1=b_in_sb[:, fb:fb + 1],
                                scalar2=None,
                                op0=mybir.AluOpType.add,
                            )
                        for kci in range(TB // P):
                            kc = tbi * (TB // P) + kci
                            tp = psum.tile([P, P], BF16, name="qkvp", bufs=2)
                            nc.tensor.transpose(
                                tp[:], v_sb[:, kci * P:(kci + 1) * P], identity[:]
                            )
                            nc.vector.tensor_copy(vT[:, b, kc, 0:64], tp[:, 0:64])
                            nc.vector.tensor_copy(vT[:, b, kc, 65:129], tp[:, 64:128])

                    def emit_scores(qb, kps, es_tiles, b=b):
                        q0 = b * S + qb * QB
                        for kp in kps:
                            for h in range(2):
                                for ki in range(2):
                                    kc = kp * 2 + ki
                                    sc = psum.tile([P, QB], F32, name="sc",
                                                   bufs=2)
                                    for qh in range(QB // QQ):
                                        nc.tensor.matmul(
                                            sc[:, qh * QQ:(qh + 1) * QQ],
                                            qkT[h * D:(h + 1) * D, 1,
                                                b * S + kc * P: b * S + (kc + 1) * P],
                                            qkT[h * D:(h + 1) * D, 0,
                                                q0 + qh * QQ: q0 + (qh + 1) * QQ],
                                            start=True, stop=True,
                                            tile_position=(h * D, 0),
                                        )
                                    if kc % 4 == 0:
                                        es_tiles[(h, kc // 4)] = es_pool.tile(
                                            [P, 4, QB], BF16, name="es"
                                        )
                                    nc.scalar.activation(
                                        es_tiles[(h, kc // 4)][:, kc % 4, :],
                                        sc[:],
                                        mybir.ActivationFunctionType.Exp,
                                        scale=0.125,
                                    )

                    # Interleave: q-block 0's scores can start as soon as the
                    # first half of this batch's QKV is done; remaining token
                    # blocks overlap with exp on the ACT engine.
                    emit_tb(0)
                    emit_tb(1)
                    es_q = {0: {}, 1: {}}
                    emit_scores(0, range(0, 4), es_q[0])
                    emit_tb(2)
                    emit_scores(0, [4, 5], es_q[0])
                    emit_tb(3)
                    emit_scores(0, [6, 7], es_q[0])

                    for qb in range(NQB):
                        q0 = b * S + qb * QB
                        attn = attn_p.tile([D, 2, QB], BF16, name="attn")
                        es_tiles = es_q[qb]
                        if qb > 0:
                            emit_scores(qb, range(KC // 2), es_tiles)
                        for h in range(2):
                            for qh in range(QB // QQ):
                                pv = psum.tile([65, QQ], F32, name="pv", bufs=2)
                                for kc in range(KC):
                                    nc.tensor.matmul(
                                        pv[:],
                                        vT[:, b, kc, h * 65:(h + 1) * 65],
                                        es_tiles[(h, kc // 4)][:, kc % 4,
                                                               qh * QQ:(qh + 1) * QQ],
                                        start=(kc == 0),
                                        stop=(kc == KC - 1),
                                    )
                                inv = small.tile([1, QQ], F32, name="inv")
                                nc.vector.reciprocal(inv[:], pv[64:65, :])
                                inv_b = small.tile([D, QQ], F32, name="invb")
                                nc.gpsimd.partition_broadcast(inv_b[:], inv[:], channels=D)
                                nc.vector.tensor_tensor(
                                    out=attn[:, h, qh * QQ:(qh + 1) * QQ],
                                    in0=pv[0:64, :],
                                    in1=inv_b[:],
                                    op=mybir.AluOpType.mult,
                                )

                        # ---- out_proj for this q block ----
                        for tci in range(QB // P):
                            tc_g = (q0 + tci * P) // P
                            out_sb = outp.tile([P, E], F32, name="osb")
                            for eb in range(E // 512):
                                op = psum.tile([P, 512], F32, name="pv", bufs=2)
                                for h in range(2):
                                    nc.tensor.matmul(
                                        op[:],
                                        attn[:, h, tci * P:(tci + 1) * P],
                                        w_outT_sb[:, h, eb * 512:(eb + 1) * 512],
                                        start=(h == 0),
                                        stop=(h == 1),
                                    )
                                nc.vector.tensor_copy(
                                    out_sb[:, eb * 512:(eb + 1) * 512], op[:]
                                )
                            nc.sync.dma_start(out_d[tc_g], out_sb[:])

    nc.compile()
    return nc


def _prep_inputs(x, w_in, b_in, w_out):
    x = np.ascontiguousarray(np.asarray(x, dtype=np.float32))
    w_in = np.asarray(w_in, dtype=np.float32)
    b_in = np.asarray(b_in, dtype=np.float32)
    w_out = np.asarray(w_out, dtype=np.float32)

    xT = np.ascontiguousarray(
        x.reshape(T, E).T.reshape(EC, P, T).transpose(1, 0, 2)
    ).astype(ml_dtypes.bfloat16)  # [128, EC, T]

    in_maps = []
    for c in range(NCORES):
        r0 = c * 2 * D  # 128*c
        rows = np.concatenate([
            w_in[0 * E + r0: 0 * E + r0 + 2 * D],
            w_in[1 * E + r0: 1 * E + r0 + 2 * D],
            w_in[2 * E + r0: 2 * E + r0 + 2 * D],
        ])                                     # [384, 1024]
        w_inT_c = np.ascontiguousarray(
            rows.T.reshape(EC, P, 3 * P).transpose(1, 0, 2)
        ).astype(ml_dtypes.bfloat16)           # [128, EC, 384]
        b_c = np.concatenate([
            b_in[0 * E + r0: 0 * E + r0 + 2 * D],
            b_in[1 * E + r0: 1 * E + r0 + 2 * D],
            b_in[2 * E + r0: 2 * E + r0 + 2 * D],
        ]).reshape(3, P).T.copy()              # [128, 3]
        w_outT_c = np.ascontiguousarray(
            w_out[:, r0: r0 + 2 * D].T.reshape(2, D, E).transpose(1, 0, 2)
        ).astype(ml_dtypes.bfloat16)           # [64, 2, 1024]
        in_maps.append({
            "xT": xT,
            "w_inT": w_inT_c,
            "b_in": b_c,
            "w_outT": w_outT_c,
        })
    return in_maps


def kernel(x, w_in, b_in, w_out, b_out, _trace=False):
    global _COMPILED
    if _COMPILED is None:
        _COMPILED = build()
    nc = _COMPILED

    in_maps = _prep_inputs(x, w_in, b_in, w_out)
    res = run_bass_kernel_spmd(
        nc, in_maps, core_ids=list(range(NCORES)), trace=_trace
    )
    partial = np.zeros((TCH, P, E), dtype=np.float32)
    for c in range(NCORES):
        partial += res.results[c]["out"]
    out = partial.reshape(T, E) + np.asarray(b_out, dtype=np.float32)
    out = out.reshape(B, S, E)
    if _trace:
        return out, res
    return out

